# revision 9
# baseline (speedup 1.0000x reference)
"""PointConvDensity forward on 8 Trainium2 NeuronCores (Bass/Tile).

Math (see reference): per (b, n, s):
    h[o] = W @ feat + bias;  feat = [pts - c, g - 2c, c, 1/(|g-c|+1e-8)]
    BN(train) over (b,n,s) per channel -> relu -> max over s.

Decomposition:
    h[o,n,s] = base[o,n] + Wu[o]*u[n,s] + Wv[o]*v[n,s]
      base  = Wb @ [points; xyz; ones]   (K=128 GEMM, weight transform on host)
      u     = g - 2c,  v = 1/(|g-c| + 1e-8),  g = xyz[idx] (host-prepped
              compact gather: tile p holds n in [16p,16p+16), s-minor)
    With q = sign(gamma) folded into the weights (qh = q*h):
      max_s relu(scale*h + shift) = relu(|scale| * (qb + max_s r2) + shift)
    BN stats from decomposed sums (no pass over the (o,n,s) cube):
      Sh  = q*(S*Sum_n qb + a*Su + b*Sv)
      Sh2 = S*Sum qb^2 + 2(a*qBsu + b*qBsv) + a^2*Suu + b^2*Svv + 2ab*Suv
    Cross-core: one 8KB AllReduce of the aggregates, overlapped with the
    main rank-2 loop.

Engine plan (v4): the (o,n,s) max-reduce is split three ways per 4-tile
psum group: 'x' = direct DVE reduce, 'y' = gpsimd TT-max halving then DVE
bf16 tree, 'z' = ACT f32->bf16 copy then DVE bf16 tree (bf16 TT gets the
DVE 2x_1p fast path; InstTensorReduce has none).  Stats sums run on the
gpsimd STT accumulator + ACT copy-accum so the DVE queue stays on the
main loop.  rmax is bf16 (error << BN tolerance).  Output is transposed
j-strided so every DRAM row burst is 8KB contiguous.
"""

import numpy as np
import ml_dtypes

B, N, S = 8, 2048, 32
OUT = 128
QT = 16              # tiles per expand chunk (2 DMAs per chunk)
BN_EPS = 1e-5
CNT = float(B * N * S)

# per-group reduce route: x=DVE direct, z=ACT bf16 copy + DVE bf16 tree
# (gpsimd cannot read PSUM, so no Pool route)
ROUTE = ['z'] * 32
for _g in (0, 2, 4, 6, 14, 22, 29):
    ROUTE[_g] = 'x'

_CACHE = {}


def _build_nc():
    import concourse.bass as bass
    import concourse.bacc as bacc
    import concourse.tile as tile
    import concourse.mybir as mybir
    from contextlib import ExitStack

    f32 = mybir.dt.float32
    f32r = mybir.dt.float32r
    bf16 = mybir.dt.bfloat16
    AF = mybir.ActivationFunctionType
    ALU = mybir.AluOpType
    AX = mybir.AxisListType

    nc = bacc.Bacc("TRN2", target_bir_lowering=False, debug=False, num_devices=8)

    d_gcw = nc.dram_tensor("gcw", [128, 512], f32, kind="ExternalInput").ap()
    d_rbf = nc.dram_tensor("rbf", [128, N], bf16, kind="ExternalInput").ap()
    d_lbf = nc.dram_tensor("lbf", [128, 128], bf16, kind="ExternalInput").ap()
    d_ab = nc.dram_tensor("ab2", [2, 128], f32r, kind="ExternalInput").ap()
    d_cc = nc.dram_tensor("cvec", [128, 16], f32, kind="ExternalInput").ap()
    d_fin = nc.dram_tensor("fin", [128, 8], f32, kind="ExternalInput").ap()
    d_id = nc.dram_tensor("ident", [128, 128], f32, kind="ExternalInput").ap()
    d_one = nc.dram_tensor("ones", [1, 128], f32r, kind="ExternalInput").ap()
    d_out = nc.dram_tensor("out", [N, OUT], f32, kind="ExternalOutput").ap()

    with tile.TileContext(nc) as tc, ExitStack() as ctx:
        sb = ctx.enter_context(tc.tile_pool(name="sb", bufs=1))
        sbuv = ctx.enter_context(tc.tile_pool(name="sbuv", bufs=3))
        sbtr = ctx.enter_context(tc.tile_pool(name="sbtr", bufs=2))
        ps = ctx.enter_context(tc.tile_pool(name="ps", bufs=2, space="PSUM"))
        dram = ctx.enter_context(tc.tile_pool(name="dram", bufs=1, space="DRAM"))

        # ---------- input loads (u/v dependencies first) ----------
        t_gc = sb.tile([128, 512], f32, name="gc")
        t_cc = sb.tile([128, 16], f32, name="cc")
        t_ab = sb.tile([2, 128], f32r, name="ab")
        t_one = sb.tile([1, 128], f32r, name="ones")
        t_fin = sb.tile([128, 8], f32, name="fin")
        t_rbf = sb.tile([128, N], bf16, name="rbf")
        t_lbf = sb.tile([128, 128], bf16, name="lbf")
        t_id = sb.tile([128, 128], f32, name="ident")
        nc.sync.dma_start(t_gc[:, :], d_gcw)
        nc.sync.dma_start(t_cc[:, :], d_cc)
        nc.sync.dma_start(t_ab[:, :], d_ab)
        nc.sync.dma_start(t_one[:, :], d_one)
        nc.sync.dma_start(t_fin[:, :], d_fin)
        nc.sync.dma_start(t_rbf[:, :], d_rbf)
        nc.sync.dma_start(t_lbf[:, :], d_lbf)
        nc.sync.dma_start(t_id[:, :], d_id)

        # ---------- u, v (DVE only; no ACT tables on this path) ----------
        uvS = sb.tile([128, 1024], f32r, name="uvS")
        t_u = uvS[:, 0:512]
        t_v = uvS[:, 512:1024]
        cc_b = t_cc[:, :].unsqueeze(2).broadcast_to([128, 16, 32])
        gc3 = t_gc[:, :].rearrange("p (j s) -> p j s", s=32)
        t_t = sb.tile([128, 512], f32, name="t_t")
        t_w = sb.tile([128, 512], f32, name="t_w")
        t3 = t_t[:, :].rearrange("p (j s) -> p j s", s=32)
        nc.vector.tensor_sub(t3, gc3, cc_b)                       # t = g - c
        nc.vector.tensor_sub(t_u.rearrange("p (j s) -> p j s", s=32), t3, cc_b)
        nc.vector.scalar_tensor_tensor(t_w[:, :], t_t[:, :], -1.0, t_t[:, :],
                                       ALU.mult, ALU.max)          # |t|
        nc.vector.tensor_scalar_add(t_w[:, :], t_w[:, :], 1e-8)
        with nc.allow_low_precision(reason="f32r bytes are f32"):
            nc.vector.reciprocal(t_v, t_w[:, :])                  # v

        # segment sums of u, v (feed qBsu broadcast rows + Su/Sv)
        t_ar = sb.tile([128, 16], f32, name="ar_in")
        nc.vector.memset(t_ar[:, :], 0.0)
        t_su = sb.tile([128, 16], f32r, name="su_seg")
        t_sv = sb.tile([128, 16], f32r, name="sv_seg")
        with nc.allow_low_precision(reason="f32r bytes are f32"):
            nc.vector.tensor_reduce(t_su[:, :],
                                    t_u.rearrange("p (j s) -> p j s", s=32),
                                    AX.X, ALU.add)
            nc.vector.tensor_reduce(t_sv[:, :],
                                    t_v.rearrange("p (j s) -> p j s", s=32),
                                    AX.X, ALU.add)
        nc.vector.tensor_reduce(t_ar[:, 4:5], t_su[:, :], AX.X, ALU.add)
        nc.vector.tensor_reduce(t_ar[:, 5:6], t_sv[:, :], AX.X, ALU.add)

        # ---------- expand + main loop plumbing ----------
        t_rmax = sb.tile([128, N], bf16, name="rmax")
        scr2 = sb.tile([128, N], f32, name="scr2")
        qb_sb = sb.tile([128, N], f32, name="qb_sb")
        uv_bufs = {}

        def expand(q):
            uv_buf = sbuv.tile([2, QT * 512], f32r, name="uvq")
            psl = slice(q * QT, (q + 1) * QT)
            for rr in range(2):
                nc.sync.dma_start(uv_buf[rr:rr + 1, :],
                                  uvS[psl, rr * 512:(rr + 1) * 512])
            uv_bufs[q] = uv_buf

        def group_mm(g):
            """4 rank-2 matmuls of group g into a fresh 4-bank psum tile."""
            q, gg = divmod(g, QT // 4)
            uv_buf = uv_bufs[q]
            r2ps = ps.tile([128, 2048], f32, name="bigps")
            for k in range(4):
                cb = gg * 4 + k
                nc.tensor.matmul(r2ps[:, k * 512:(k + 1) * 512],
                                 t_ab[:, :],
                                 uv_buf[:, cb * 512:(cb + 1) * 512],
                                 start=True, stop=True)
            return r2ps

        def tree(g, src_bf):
            """bf16 TT-max tree from [128, 64, 16] down to rmax[:, g*64:]."""
            out_sl = t_rmax[:, g * 64:(g + 1) * 64]
            tB = sbtr.tile([128, 512], bf16, name="trB")
            tC = sbtr.tile([128, 256], bf16, name="trC")
            tD = sbtr.tile([128, 128], bf16, name="trD")
            B3 = tB[:, :].rearrange("p (j s) -> p j s", s=8)
            C3 = tC[:, :].rearrange("p (j s) -> p j s", s=4)
            D3 = tD[:, :].rearrange("p (j s) -> p j s", s=2)
            nc.vector.tensor_max(B3, src_bf[:, :, 0:8], src_bf[:, :, 8:16])
            nc.vector.tensor_max(C3, B3[:, :, 0:4], B3[:, :, 4:8])
            nc.vector.tensor_max(D3, C3[:, :, 0:2], C3[:, :, 2:4])
            nc.vector.tensor_max(out_sl.rearrange("p (j s) -> p j s", s=1),
                                 D3[:, :, 0:1], D3[:, :, 1:2])

        def group_reduce(g, r2ps):
            ps3 = r2ps[:, :].rearrange("p (j s) -> p j s", s=32)
            route = ROUTE[g]
            if route == 'x':
                nc.vector.tensor_reduce(t_rmax[:, g * 64:(g + 1) * 64],
                                        ps3, AX.X, ALU.max)
            else:  # 'z'
                tZ = sbtr.tile([128, 2048], bf16, name="trZ")
                nc.scalar.copy(tZ[:, :], r2ps[:, :])
                Z3 = tZ[:, :].rearrange("p (j s) -> p j s", s=32)
                tA = sbtr.tile([128, 1024], bf16, name="trA")
                A3 = tA[:, :].rearrange("p (j s) -> p j s", s=16)
                nc.vector.tensor_max(A3, Z3[:, :, 0:16], Z3[:, :, 16:32])
                tree(g, A3)

        def run_group(g):
            group_reduce(g, group_mm(g))

        # chunks 0-2 prefetched, groups 0-7 keep PE/DVE busy from ~4us
        expand(0)
        expand(1)
        for g in range(0, 4):
            run_group(g)
        expand(2)
        for g in range(4, 8):
            run_group(g)

        # ---------- base GEMM (bf16) + stats (Pool STT accum + ACT) ------
        qb_ps = ps.tile([128, 2048], f32, name="bigps")
        for j in range(4):
            sl = slice(j * 512, (j + 1) * 512)
            nc.tensor.matmul(qb_ps[:, sl], t_lbf[:, :], t_rbf[:, sl],
                             start=True, stop=True)
        nc.scalar.copy(qb_sb[:, :], qb_ps[:, :])

        # ACT-accumulated sums: Suu, Svv, Suv (sink -> sink_p)
        sink_p = sb.tile([128, 512], f32, name="sink_p")
        nc.scalar.activation(sink_p[:, :], t_u, AF.Square,
                             accum_out=t_ar[:, 6:7])
        nc.scalar.activation(sink_p[:, :], t_v, AF.Square,
                             accum_out=t_ar[:, 7:8])
        nc.vector.tensor_mul(t_w[:, :], t_u, t_v)
        nc.scalar.activation(sink_p[:, :], t_w[:, :], AF.Copy,
                             accum_out=t_ar[:, 8:9])
        # ACT: qb sum + sum of squares (Copy/Square accum)
        nc.scalar.activation(scr2[:, :], qb_sb[:, :], AF.Copy,
                             accum_out=t_ar[:, 0:1])
        nc.scalar.activation(scr2[:, :], qb_sb[:, :], AF.Square,
                             accum_out=t_ar[:, 1:2])

        # qBsu / qBsv: K=1 broadcast matmuls + Pool STT accum
        t_rows = sb.tile([1, 2 * N], f32r, name="t_rows")
        t_sur = t_rows[:, 0:N]
        t_svr = t_rows[:, N:2 * N]
        nc.sync.dma_start(t_sur.rearrange("o (p j) -> o p j", j=16), t_su[:, :])
        nc.sync.dma_start(t_svr.rearrange("o (p j) -> o p j", j=16), t_sv[:, :])
        for ci, (t_row, acol) in enumerate(((t_sur, 2), (t_svr, 3))):
            bc_ps = ps.tile([128, 2048], f32, name="bigps")
            for j in range(4):
                sl = slice(j * 512, (j + 1) * 512)
                nc.tensor.matmul(bc_ps[:, sl], t_one[:, :], t_row[:, sl],
                                 start=True, stop=True)
            nc.vector.scalar_tensor_tensor(scr2[:, :], qb_sb[:, :], 1.0,
                                           bc_ps[:, :], ALU.mult, ALU.mult,
                                           accum_out=t_ar[:, acol:acol + 1])

        # ---------- AllReduce of aggregates (overlaps the main loop) ----
        arA = dram.tile([128, 16], f32, name="arA")
        arB = dram.tile([128, 16], f32, name="arB")
        nc.gpsimd.dma_start(arA[:, :], t_ar[:, :])
        nc.gpsimd.collective_compute(
            "AllReduce", ALU.add,
            replica_groups=[list(range(8))],
            ins=[arA[:, :].opt()],
            outs=[arB[:, :].opt()],
        )

        # ---------- main loop body (collective overlapped) ----------
        expand(3)
        for g in range(8, 12):
            run_group(g)
        expand(4)
        for g in range(12, 16):
            run_group(g)
        expand(5)
        for g in range(16, 20):
            run_group(g)
        expand(6)
        for g in range(20, 24):
            run_group(g)

        # collective readback + cross-partition totals (Pool queue is past
        # the early TTs by now; Comms finished long ago)
        t_arg = sb.tile([128, 16], f32, name="ar_out")
        nc.gpsimd.dma_start(t_arg[:, :], arB[:, :])
        t_red1 = sb.tile([1, 8], f32, name="red1")
        nc.gpsimd.tensor_reduce(t_red1[:, 0:5], t_arg[:, 4:9], AX.C, ALU.add)
        t_one32 = sb.tile([1, 128], f32, name="ones32")
        nc.vector.memset(t_one32[:, :], 1.0)
        red_ps = ps.tile([128, 2048], f32, name="bigps")
        nc.tensor.matmul(red_ps[:, 0:8], t_one32[:, :], t_red1[:, :],
                         start=True, stop=True)
        t_red = sb.tile([128, 8], f32, name="ar_red")
        nc.scalar.copy(t_red[:, :], red_ps[:, 0:8])

        expand(7)
        for g in range(24, 28):
            run_group(g)
        for g in range(28, 30):
            run_group(g)

        # ---------- finalize scale/shift (ready well before loop end) ----
        def col(t, i):
            return t[:, i:i + 1]

        a_, b_ = col(t_fin, 0), col(t_fin, 1)
        gab, bet = col(t_fin, 2), col(t_fin, 3)
        f1 = sb.tile([128, 12], f32, name="fwork")
        nc.vector.tensor_scalar_mul(col(f1, 0), col(t_arg, 0), float(S))
        nc.vector.tensor_mul(col(f1, 1), a_, col(t_red, 0))
        nc.vector.tensor_mul(col(f1, 2), b_, col(t_red, 1))
        nc.vector.tensor_add(col(f1, 0), col(f1, 0), col(f1, 1))
        nc.vector.tensor_add(col(f1, 0), col(f1, 0), col(f1, 2))   # Sh_pre
        nc.vector.tensor_scalar_mul(col(f1, 3), col(t_arg, 1), float(S))
        nc.vector.tensor_mul(col(f1, 4), a_, col(t_arg, 2))
        nc.vector.tensor_mul(col(f1, 5), b_, col(t_arg, 3))
        nc.vector.tensor_add(col(f1, 4), col(f1, 4), col(f1, 5))
        nc.vector.tensor_scalar_mul(col(f1, 4), col(f1, 4), 2.0)
        nc.vector.tensor_add(col(f1, 3), col(f1, 3), col(f1, 4))
        nc.vector.tensor_mul(col(f1, 5), a_, a_)
        nc.vector.tensor_mul(col(f1, 5), col(f1, 5), col(t_red, 2))
        nc.vector.tensor_add(col(f1, 3), col(f1, 3), col(f1, 5))
        nc.vector.tensor_mul(col(f1, 5), b_, b_)
        nc.vector.tensor_mul(col(f1, 5), col(f1, 5), col(t_red, 3))
        nc.vector.tensor_add(col(f1, 3), col(f1, 3), col(f1, 5))
        nc.vector.tensor_mul(col(f1, 5), a_, b_)
        nc.vector.tensor_mul(col(f1, 5), col(f1, 5), col(t_red, 4))
        nc.vector.tensor_scalar_mul(col(f1, 5), col(f1, 5), 2.0)
        nc.vector.tensor_add(col(f1, 3), col(f1, 3), col(f1, 5))   # Sh2
        nc.vector.tensor_scalar_mul(col(f1, 6), col(f1, 0), 1.0 / CNT)
        nc.vector.tensor_mul(col(f1, 7), col(f1, 6), col(f1, 6))
        nc.vector.tensor_scalar_mul(col(f1, 8), col(f1, 3), 1.0 / CNT)
        nc.vector.tensor_sub(col(f1, 8), col(f1, 8), col(f1, 7))   # var
        t_epsbn = sb.tile([128, 1], f32, name="epsbn")
        nc.vector.memset(t_epsbn[:, :], BN_EPS)
        nc.scalar.activation(col(f1, 9), col(f1, 8), AF.Sqrt, bias=t_epsbn[:, :])
        t_rs = sb.tile([128, 1], f32, name="rs")
        nc.vector.reciprocal(t_rs[:, :], col(f1, 9))
        t_asc = sb.tile([128, 1], f32, name="ascale")
        t_shf = sb.tile([128, 1], f32, name="shift")
        nc.vector.tensor_mul(t_asc[:, :], gab, t_rs[:, :])
        nc.vector.tensor_mul(t_shf[:, :], col(f1, 6), t_asc[:, :])
        nc.vector.tensor_sub(t_shf[:, :], bet, t_shf[:, :])

        for g in range(30, 32):
            run_group(g)

        # ---------- tail: j-strided add/relu/transpose pipeline ----------
        # out rows n = 16p + j live on partition p: 8KB contiguous bursts.
        t_o = sb.tile([128, N], f32, name="ot")
        t_ot = sb.tile([128, 16 * 128], f32, name="otT")
        for j in range(16):
            mj = scr2[:, j::16]
            nc.vector.scalar_tensor_tensor(mj, t_rmax[:, j::16], 1.0,
                                           qb_sb[:, j::16], ALU.mult, ALU.add)
            nc.scalar.activation(t_o[:, j::16], mj, AF.Relu,
                                 bias=t_shf[:, :], scale=t_asc[:, :])
            tp_ps = ps.tile([128, 2048], f32, name="bigps")
            nc.tensor.transpose(tp_ps[:, 0:128], t_o[:, j::16], t_id[:, :])
            if j % 2 == 0:
                nc.scalar.copy(t_ot[:, j * 128:(j + 1) * 128], tp_ps[:, 0:128])
            else:
                nc.vector.tensor_scalar_mul(t_ot[:, j * 128:(j + 1) * 128],
                                            tp_ps[:, 0:128], 1.0)
        nc.sync.dma_start(d_out.rearrange("(p j) o -> p j o", j=16),
                          t_ot[:, :].rearrange("p (j o) -> p j o", o=128))

    nc.compile()
    return nc


def _get_nc():
    if "nc" not in _CACHE:
        _CACHE["nc"] = _build_nc()
    return _CACHE["nc"]


def _prep_inputs(xyz, points, idx, W, b, gamma, beta):
    xyz = np.asarray(xyz, np.float32)
    points = np.asarray(points, np.float32)
    idx = np.asarray(idx).astype(np.int64)
    W = np.asarray(W, np.float32)
    b = np.asarray(b, np.float32)
    gamma = np.asarray(gamma, np.float32)
    beta = np.asarray(beta, np.float32)

    D = points.shape[1]
    q = np.where(gamma >= 0, np.float32(1.0), np.float32(-1.0))
    Wpts = W[:, :D]
    Wu = W[:, D]
    Wc = W[:, D + 1] - Wpts.sum(axis=1)
    Wv = W[:, D + 2]
    lhsb = np.zeros((128, 128), np.float32)
    lhsb[:D, :] = q[None, :] * Wpts.T
    lhsb[126, :] = q * Wc
    lhsb[127, :] = q * b
    lbf16 = lhsb.astype(ml_dtypes.bfloat16)

    a_ = (q * Wu).astype(np.float32)
    b_ = (q * Wv).astype(np.float32)
    ab2 = np.stack([a_, b_], axis=0)          # [2, 128]

    fin = np.zeros((128, 8), np.float32)
    fin[:, 0] = a_
    fin[:, 1] = b_
    fin[:, 2] = np.abs(gamma)
    fin[:, 3] = beta

    ident = np.eye(128, dtype=np.float32)

    in_maps = []
    for bb in range(B):
        rbf = np.concatenate(
            [points[bb], xyz[bb], np.ones((1, N), np.float32)], axis=0)
        gcw = xyz[bb, 0][idx[bb]].reshape(128, 512).astype(np.float32)
        m = {
            "rbf": rbf.astype(ml_dtypes.bfloat16),
            "lbf": lbf16,
            "gcw": np.ascontiguousarray(gcw),
            "ab2": ab2,
            "cvec": np.ascontiguousarray(xyz[bb].reshape(128, 16)),
            "fin": fin,
            "ident": ident,
            "ones": np.ones((1, 128), np.float32),
        }
        in_maps.append(m)
    return in_maps


def kernel(xyz, points, idx, W, b, gamma, beta, _trace=False):
    from concourse.bass_utils import run_bass_kernel_spmd

    nc = _get_nc()
    in_maps = _prep_inputs(xyz, points, idx, W, b, gamma, beta)
    res = run_bass_kernel_spmd(nc, in_maps, core_ids=list(range(8)),
                               trace=_trace)
    if _trace:
        _CACHE["last_results"] = res
    out = np.stack([res.results[c]["out"] for c in range(8)], axis=0)
    return out


# revision 10
# speedup vs baseline: 2.3111x; 2.3111x over previous
"""PointConvDensity forward on 8 Trainium2 NeuronCores (Bass/Tile).

Math (see reference): per (b, n, s):
    h[o] = W @ feat + bias;  feat = [pts - c, g - 2c, c, 1/(|g-c|+1e-8)]
    BN(train) over (b,n,s) per channel -> relu -> max over s.

Decomposition:
    h[o,n,s] = base[o,n] + Wu[o]*u[n,s] + Wv[o]*v[n,s]
      base  = Wb @ [points; xyz; ones]   (K=128 GEMM, weight transform on host)
      u     = g - 2c,  v = 1/(|g-c| + 1e-8),  g = xyz[idx]
    With q = sign(gamma) folded into the weights (qh = q*h):
      max_s relu(scale*h + shift) = relu(|scale| * (qb + max_s r2) + shift)
    BN stats from decomposed sums (no pass over the (o,n,s) cube):
      Sh  = q*(S*Sum_n qb + a*Su + b*Sv)
      Sh2 = S*Sum qb^2 + 2(a*qBsu + b*qBsv) + a^2*Suu + b^2*Svv + 2ab*Suv
    where a=q*Wu, b=q*Wv and qBsu[o] = Sum_n qb[o,n]*su[n], su = Sum_s u.
    Cross-core: one 8KB AllReduce of the aggregates, overlapped with the
    main rank-2 loop.

v2 changes vs v1:
  - gather via InstIndirectCopy (resident firmware) instead of ap_gather:
    avoids the ~223us GPSIMD library swap that dominated v1.
  - partition_all_reduce (attn library) replaced by a lib-0 gpsimd C-axis
    reduce + K=1 matmul broadcast: avoids the second ~25us library swap.
  - all matmuls run in fp32r (1 cycle/row at >=256 cols): no bf16 split-K
    machinery, 2-row rank-2 rhs instead of 12 rows.
  - main loop writes 4 matmuls into one 4-bank PSUM tile, one big DVE
    reduce each: fewer instructions, less PSUM-access overhead.
  - program order keeps the AllReduce + finalize off the critical path
    (expand DMAs issue before the collective-dependent readback).
"""

import numpy as np

B, N, S = 8, 2048, 32
OUT = 128
QT = 16              # tiles per expand chunk (2 DMAs per chunk)
BN_EPS = 1e-5
CNT = float(B * N * S)

_CACHE = {}


def _build_nc():
    import concourse.bass as bass
    import concourse.bacc as bacc
    import concourse.tile as tile
    import concourse.mybir as mybir
    from contextlib import ExitStack

    f32 = mybir.dt.float32
    f32r = mybir.dt.float32r
    u16 = mybir.dt.uint16
    AF = mybir.ActivationFunctionType
    ALU = mybir.AluOpType
    AX = mybir.AxisListType

    nc = bacc.Bacc("TRN2", target_bir_lowering=False, debug=False, num_devices=8)

    # ---- DRAM I/O (per-core shapes) ----
    d_gcw = nc.dram_tensor("gcw", [128, 512], f32, kind="ExternalInput").ap()
    d_rbf = nc.dram_tensor("rbf", [128, N], f32r, kind="ExternalInput").ap()
    d_lbf = nc.dram_tensor("lbf", [128, 128], f32r, kind="ExternalInput").ap()
    d_ab = nc.dram_tensor("ab2", [2, 128], f32r, kind="ExternalInput").ap()
    d_cc = nc.dram_tensor("cvec", [128, 16], f32, kind="ExternalInput").ap()
    d_fin = nc.dram_tensor("fin", [128, 8], f32, kind="ExternalInput").ap()
    d_id = nc.dram_tensor("ident", [128, 128], f32, kind="ExternalInput").ap()
    d_one = nc.dram_tensor("ones", [1, 128], f32r, kind="ExternalInput").ap()
    d_out = nc.dram_tensor("out", [N, OUT], f32, kind="ExternalOutput").ap()

    with tile.TileContext(nc) as tc, ExitStack() as ctx:
        sb = ctx.enter_context(tc.tile_pool(name="sb", bufs=1))
        sb2 = ctx.enter_context(tc.tile_pool(name="sb2", bufs=2))
        ps = ctx.enter_context(tc.tile_pool(name="ps", bufs=2, space="PSUM"))
        dram = ctx.enter_context(tc.tile_pool(name="dram", bufs=1, space="DRAM"))

        # ---------- load inputs (u/v deps first) ----------
        t_gc = sb.tile([128, 512], f32, name="gc")
        t_rbf = sb.tile([128, N], f32r, name="rbf")
        t_lbf = sb.tile([128, 128], f32r, name="lbf")
        t_ab = sb.tile([2, 128], f32r, name="ab")
        t_cc = sb.tile([128, 16], f32, name="cc")
        t_fin = sb.tile([128, 8], f32, name="fin")
        t_id = sb.tile([128, 128], f32, name="ident")
        nc.sync.dma_start(t_gc[:, :], d_gcw)
        nc.sync.dma_start(t_rbf[:, :], d_rbf)
        nc.sync.dma_start(t_lbf[:, :], d_lbf)
        nc.sync.dma_start(t_ab[:, :], d_ab)
        nc.sync.dma_start(t_cc[:, :], d_cc)
        nc.sync.dma_start(t_fin[:, :], d_fin)
        nc.sync.dma_start(t_id[:, :], d_id)

        # ---------- u, v on the compact layout ----------
        # uvS[p, 0:512] = u rows, uvS[p, 512:1024] = v rows (f32, fp32r matmul)
        uvS = sb.tile([128, 1024], f32r, name="uvS")
        t_u = uvS[:, 0:512]
        t_v = uvS[:, 512:1024]
        cc_b = t_cc[:, :].unsqueeze(2).broadcast_to([128, 16, 32])
        gc3 = t_gc[:, :].rearrange("p (j s) -> p j s", s=32)
        t_t = sb.tile([128, 512], f32, name="t_t")
        t_w = sb.tile([128, 512], f32, name="t_w")
        t3 = t_t[:, :].rearrange("p (j s) -> p j s", s=32)
        nc.vector.tensor_sub(t3, gc3, cc_b)                       # t = g - c
        nc.vector.tensor_sub(t_u.rearrange("p (j s) -> p j s", s=32), t3, cc_b)
        t_eps = sb.tile([128, 1], f32, name="eps8")
        nc.vector.memset(t_eps[:, :], 1e-8)
        nc.scalar.activation(t_w[:, :], t_t[:, :], AF.Abs)        # |t|
        t_w2 = sb.tile([128, 512], f32, name="t_w2")
        nc.scalar.activation(t_w2[:, :], t_w[:, :], AF.Identity, bias=t_eps[:, :])
        with nc.allow_low_precision(reason="f32r bytes are f32"):
            nc.vector.reciprocal(t_v, t_w2[:, :])                 # v = 1/(|t|+eps)

        # ---------- per-core stats ----------
        t_ar = sb.tile([128, 16], f32, name="ar_in")
        nc.vector.memset(t_ar[:, :], 0.0)
        u3v = t_u.rearrange("p (j s) -> p j s", s=32)
        v3v = t_v.rearrange("p (j s) -> p j s", s=32)
        t_su = sb.tile([128, 16], f32r, name="su_seg")
        t_sv = sb.tile([128, 16], f32r, name="sv_seg")
        with nc.allow_low_precision(reason="f32r bytes are f32"):
            nc.vector.tensor_reduce(t_su[:, :], u3v, AX.X, ALU.add)
            nc.vector.tensor_reduce(t_sv[:, :], v3v, AX.X, ALU.add)
        nc.vector.tensor_reduce(t_ar[:, 4:5], t_su[:, :], AX.X, ALU.add)
        nc.vector.tensor_reduce(t_ar[:, 5:6], t_sv[:, :], AX.X, ALU.add)
        # sums of squares / products via ACT accumulator (TTR broken on HW)
        sink_a = sb.tile([128, 512], f32, name="sink_a")
        nc.scalar.activation(sink_a[:, :], t_u, AF.Square, accum_out=t_ar[:, 6:7])
        nc.scalar.activation(sink_a[:, :], t_v, AF.Square, accum_out=t_ar[:, 7:8])
        scr2 = sb.tile([128, N], f32, name="scr2")
        scr = scr2[:, 0:512]
        nc.vector.tensor_mul(scr, t_u, t_v)
        nc.scalar.activation(sink_a[:, :], scr, AF.Copy, accum_out=t_ar[:, 8:9])

        # ---------- base GEMM (fp32r): qb = lbf.T @ rbf ----------
        qb_sb = sb.tile([128, N], f32, name="qb_sb")
        qb_ps = ps.tile([128, 2048], f32, name="bigps")
        for j in range(4):
            sl = slice(j * 512, (j + 1) * 512)
            nc.tensor.matmul(qb_ps[:, sl], t_lbf[:, :], t_rbf[:, sl],
                             start=True, stop=True)
        nc.scalar.copy(qb_sb[:, :], qb_ps[:, :])

        # qb row sums / row sums of squares
        nc.scalar.activation(scr2[:, :], qb_sb[:, :], AF.Copy,
                             accum_out=t_ar[:, 0:1])
        nc.scalar.activation(scr2[:, :], qb_sb[:, :], AF.Square,
                             accum_out=t_ar[:, 1:2])

        # qBsu / qBsv: su broadcast across partitions via K=1 matmul, acc ACT
        t_rows = sb.tile([1, 2 * N], f32r, name="t_rows")
        t_sur = t_rows[:, 0:N]
        t_svr = t_rows[:, N:2 * N]
        nc.sync.dma_start(t_sur.rearrange("o (p j) -> o p j", j=16), t_su[:, :])
        nc.sync.dma_start(t_svr.rearrange("o (p j) -> o p j", j=16), t_sv[:, :])
        t_one = sb.tile([1, 128], f32r, name="ones")
        nc.sync.dma_start(t_one[:, :], d_one)
        qB_part = sb.tile([128, 8], f32, name="qB_part")
        bc_ps = ps.tile([128, 2048], f32, name="bigps")
        for ci, (t_row, col) in enumerate(((t_sur, 2), (t_svr, 3))):
            for j in range(4):
                sl = slice(j * 512, (j + 1) * 512)
                nc.tensor.matmul(bc_ps[:, sl], t_one[:, :], t_row[:, sl],
                                 start=True, stop=True)
                nc.vector.tensor_mul(scr, qb_sb[:, sl], bc_ps[:, sl])
                nc.scalar.activation(sink_a[:, :], scr, AF.Copy,
                                     accum_out=qB_part[:, ci * 4 + j:ci * 4 + j + 1])
            nc.vector.tensor_reduce(t_ar[:, col:col + 1],
                                    qB_part[:, ci * 4:ci * 4 + 4], AX.X, ALU.add)

        # ---------- AllReduce of aggregates (overlaps the main loop) ----------
        arA = dram.tile([128, 16], f32, name="arA")
        arB = dram.tile([128, 16], f32, name="arB")
        nc.gpsimd.dma_start(arA[:, :], t_ar[:, :])
        nc.gpsimd.collective_compute(
            "AllReduce", ALU.add,
            replica_groups=[list(range(8))],
            ins=[arA[:, :].opt()],
            outs=[arB[:, :].opt()],
        )

        # ---------- main loop: expand -> K=2 fp32r matmul -> segmented max ----
        t_rmax = sb.tile([128, N], f32, name="rmax")
        for q in range(128 // QT):
            uv_buf = sb2.tile([2, QT * 512], f32r, name="uvq")
            psl = slice(q * QT, (q + 1) * QT)
            for rr in range(2):
                nc.sync.dma_start(uv_buf[rr:rr + 1, :],
                                  uvS[psl, rr * 512:(rr + 1) * 512])
            for gg in range(QT // 4):
                g0 = q * QT + gg * 4          # first tile of this psum group
                r2ps = ps.tile([128, 2048], f32, name="bigps")
                for k in range(4):
                    cb = gg * 4 + k
                    nc.tensor.matmul(r2ps[:, k * 512:(k + 1) * 512],
                                     t_ab[:, :],
                                     uv_buf[:, cb * 512:(cb + 1) * 512],
                                     start=True, stop=True)
                nc.vector.tensor_reduce(
                    t_rmax[:, g0 * 16:(g0 + 4) * 16],
                    r2ps[:, :].rearrange("p (j s) -> p j s", s=32),
                    AX.X, ALU.max)

        # ---------- collective readback + cross-partition totals ----------
        t_arg = sb.tile([128, 16], f32, name="ar_out")
        nc.gpsimd.dma_start(t_arg[:, :], arB[:, :])
        # partition totals of Su,Sv,Suu,Svv,Suv via lib-0 gpsimd C-reduce
        t_red1 = sb.tile([1, 8], f32, name="red1")
        nc.gpsimd.tensor_reduce(t_red1[:, 0:5], t_arg[:, 4:9], AX.C, ALU.add)
        # broadcast back to all partitions via K=1 matmul
        t_one32 = sb.tile([1, 128], f32, name="ones32")
        nc.vector.memset(t_one32[:, :], 1.0)
        red_ps = ps.tile([128, 2048], f32, name="bigps")
        nc.tensor.matmul(red_ps[:, 0:8], t_one32[:, :], t_red1[:, :],
                         start=True, stop=True)
        t_red = sb.tile([128, 8], f32, name="ar_red")
        nc.scalar.copy(t_red[:, :], red_ps[:, 0:8])

        # ---------- finalize scale/shift ----------
        def col(t, i):
            return t[:, i:i + 1]

        a_, b_ = col(t_fin, 0), col(t_fin, 1)
        gab, bet = col(t_fin, 2), col(t_fin, 3)
        f1 = sb.tile([128, 12], f32, name="fwork")
        # Sh_pre = S*ar0 + a*Su + b*Sv
        nc.vector.tensor_scalar_mul(col(f1, 0), col(t_arg, 0), float(S))
        nc.vector.tensor_mul(col(f1, 1), a_, col(t_red, 0))
        nc.vector.tensor_mul(col(f1, 2), b_, col(t_red, 1))
        nc.vector.tensor_add(col(f1, 0), col(f1, 0), col(f1, 1))
        nc.vector.tensor_add(col(f1, 0), col(f1, 0), col(f1, 2))   # f1[0] = Sh_pre
        # Sh2 = S*ar1 + 2(a*qBsu + b*qBsv) + a^2*Suu + b^2*Svv + 2ab*Suv
        nc.vector.tensor_scalar_mul(col(f1, 3), col(t_arg, 1), float(S))
        nc.vector.tensor_mul(col(f1, 4), a_, col(t_arg, 2))
        nc.vector.tensor_mul(col(f1, 5), b_, col(t_arg, 3))
        nc.vector.tensor_add(col(f1, 4), col(f1, 4), col(f1, 5))
        nc.vector.tensor_scalar_mul(col(f1, 4), col(f1, 4), 2.0)
        nc.vector.tensor_add(col(f1, 3), col(f1, 3), col(f1, 4))
        nc.vector.tensor_mul(col(f1, 5), a_, a_)
        nc.vector.tensor_mul(col(f1, 5), col(f1, 5), col(t_red, 2))
        nc.vector.tensor_add(col(f1, 3), col(f1, 3), col(f1, 5))
        nc.vector.tensor_mul(col(f1, 5), b_, b_)
        nc.vector.tensor_mul(col(f1, 5), col(f1, 5), col(t_red, 3))
        nc.vector.tensor_add(col(f1, 3), col(f1, 3), col(f1, 5))
        nc.vector.tensor_mul(col(f1, 5), a_, b_)
        nc.vector.tensor_mul(col(f1, 5), col(f1, 5), col(t_red, 4))
        nc.vector.tensor_scalar_mul(col(f1, 5), col(f1, 5), 2.0)
        nc.vector.tensor_add(col(f1, 3), col(f1, 3), col(f1, 5))   # f1[3] = Sh2
        # meanq, var, rs, ascale, shift
        nc.vector.tensor_scalar_mul(col(f1, 6), col(f1, 0), 1.0 / CNT)   # meanq
        nc.vector.tensor_mul(col(f1, 7), col(f1, 6), col(f1, 6))
        nc.vector.tensor_scalar_mul(col(f1, 8), col(f1, 3), 1.0 / CNT)
        nc.vector.tensor_sub(col(f1, 8), col(f1, 8), col(f1, 7))         # var
        t_epsbn = sb.tile([128, 1], f32, name="epsbn")
        nc.vector.memset(t_epsbn[:, :], BN_EPS)
        nc.scalar.activation(col(f1, 9), col(f1, 8), AF.Sqrt, bias=t_epsbn[:, :])
        t_rs = sb.tile([128, 1], f32, name="rs")
        nc.vector.reciprocal(t_rs[:, :], col(f1, 9))
        t_asc = sb.tile([128, 1], f32, name="ascale")
        t_shf = sb.tile([128, 1], f32, name="shift")
        nc.vector.tensor_mul(t_asc[:, :], gab, t_rs[:, :])
        nc.vector.tensor_mul(t_shf[:, :], col(f1, 6), t_asc[:, :])
        nc.vector.tensor_sub(t_shf[:, :], bet, t_shf[:, :])

        # ---------- m = qb + rmax; out = relu(ascale*m + shift); transpose ----
        t_m = scr2
        nc.vector.tensor_add(t_m[:, :], qb_sb[:, :], t_rmax[:, :])
        t_o = sb.tile([128, N], f32, name="ot", tag="rbf")
        nc.scalar.activation(t_o[:, :], t_m[:, :], AF.Relu,
                             bias=t_shf[:, :], scale=t_asc[:, :])
        t_ot = sb.tile([128, 16 * 128], f32, name="otT")
        for c in range(16):
            tp_ps = ps.tile([128, 2048], f32, name="bigps")
            nc.tensor.transpose(tp_ps[:, 0:128], t_o[:, c * 128:(c + 1) * 128],
                                t_id[:, :])
            nc.scalar.copy(t_ot[:, c * 128:(c + 1) * 128], tp_ps[:, 0:128])
        # out[n, o] with n = 128*c + p  ->  one DMA
        nc.sync.dma_start(d_out.rearrange("(c p) o -> p c o", p=128),
                          t_ot[:, :].rearrange("p (c o) -> p c o", o=128))

    nc.compile()
    return nc


def _get_nc():
    if "nc" not in _CACHE:
        _CACHE["nc"] = _build_nc()
    return _CACHE["nc"]


def _prep_inputs(xyz, points, idx, W, b, gamma, beta):
    xyz = np.asarray(xyz, np.float32)
    points = np.asarray(points, np.float32)
    idx = np.asarray(idx).astype(np.int64)
    W = np.asarray(W, np.float32)
    b = np.asarray(b, np.float32)
    gamma = np.asarray(gamma, np.float32)
    beta = np.asarray(beta, np.float32)

    D = points.shape[1]
    q = np.where(gamma >= 0, np.float32(1.0), np.float32(-1.0))
    Wpts = W[:, :D]
    Wu = W[:, D]
    Wc = W[:, D + 1] - Wpts.sum(axis=1)
    Wv = W[:, D + 2]
    lhsb = np.zeros((128, 128), np.float32)
    lhsb[:D, :] = q[None, :] * Wpts.T
    lhsb[126, :] = q * Wc
    lhsb[127, :] = q * b

    a_ = (q * Wu).astype(np.float32)
    b_ = (q * Wv).astype(np.float32)
    ab2 = np.stack([a_, b_], axis=0)          # [2, 128]

    fin = np.zeros((128, 8), np.float32)
    fin[:, 0] = a_
    fin[:, 1] = b_
    fin[:, 2] = np.abs(gamma)
    fin[:, 3] = beta

    ident = np.eye(128, dtype=np.float32)

    in_maps = []
    for bb in range(B):
        rbf = np.concatenate(
            [points[bb], xyz[bb], np.ones((1, N), np.float32)], axis=0)
        gcw = xyz[bb, 0][idx[bb]].reshape(128, 512).astype(np.float32)
        m = {
            "rbf": np.ascontiguousarray(rbf),
            "lbf": lhsb,
            "gcw": np.ascontiguousarray(gcw),
            "ab2": ab2,
            "cvec": np.ascontiguousarray(xyz[bb].reshape(128, 16)),
            "fin": fin,
            "ident": ident,
            "ones": np.ones((1, 128), np.float32),
        }
        in_maps.append(m)
    return in_maps


def kernel(xyz, points, idx, W, b, gamma, beta, _trace=False):
    from concourse.bass_utils import run_bass_kernel_spmd

    nc = _get_nc()
    in_maps = _prep_inputs(xyz, points, idx, W, b, gamma, beta)
    res = run_bass_kernel_spmd(nc, in_maps, core_ids=list(range(8)),
                               trace=_trace)
    if _trace:
        _CACHE["last_results"] = res
    out = np.stack([res.results[c]["out"] for c in range(8)], axis=0)
    return out
